# revision 26
# baseline (speedup 1.0000x reference)
"""Block-sparse flash attention (local + vertical-stride pattern) on 8 TRN2
NeuronCores.

Sharding: tensor-parallel over heads. Core c gets q-heads [4c, 4c+4) and
kv-head c (the GQA group maps exactly: q-head h uses kv-head h//4). No
collectives needed; outputs are concatenated along the feature dim on host.

Per-core kernel (all shapes static, fully unrolled):
  - q is processed in tiles of 256 rows (4 sparse blocks of 64).
  - Scores are computed transposed, S^T[kv, q], per 128-wide kv chunk:
      matmul(out=S^T chunk, lhsT=K^T[:, kv_chunk], rhs=Q^T[:, q_tile])
    so softmax never needs a transpose of P for the PV matmul.
  - The window's leading staircase chunks are computed only at their live
    causal-prefix widths (64/192 q cols); remote (vertical-stride) blocks
    are packed by the HOST into per-(head,tile) staging chunks (KS/VS), so
    the SPMD program is core-independent and no wasted remote-parity
    columns are computed or exp'd.
  - exp (with the 1/sqrt(d) scale folded in) runs on the scalar engine
    straight out of PSUM into SBUF bf16, one instruction per PSUM group
    tile (6 kv chunks) so score matmuls of the next tile can reuse PSUM
    while exp of the previous group is still running.
  - Masking is multiplicative on P^T after exp: one combined causal mask
    for the adjacent D0+D1 diagonal chunks, a per-(head,tile) data mask
    for the staircase prefix chunks, and a 0/1 per-partition selector for
    dead staging slots.
  - PV accumulates O[q, d] with lhsT=P^T chunk and rhs=[V | 1]; the ones
    column makes the softmax denominator fall out of the same matmuls.
  - Emission is software-pipelined at group granularity: the PV matmul
    halves of tile t-1 are woven between the score-group matmuls of tile
    t, and within a group wide streams go first so every LDWEIGHTS hides
    under the previous matmul's rhs stream.
"""

import numpy as np
import ml_dtypes

BF16 = ml_dtypes.bfloat16

# Problem constants (hardcoded; see module docstring).
S = 2048
NUM_HEADS = 32
NUM_KV_HEADS = 8
D = 128
BLK = 64
LOCAL_BLOCKS = 16
VSTRIDE = 8
SCALE = 0.08838834764831845
NCORES = 8
HPC = NUM_HEADS // NCORES          # heads per core = 4
QTILE = 256                        # q rows per tile (4 sparse blocks)
NT = S // QTILE                    # 8 tiles
NCHUNK = S // 128                  # 16 kv chunks of 128
GROUP = 6                          # kv chunks per PSUM group tile (3 banks)


def _tile_plan(j, t):
    """Static slot plan for head-slot j (0..3), q-tile t. Core-independent.

    Returns a list of slots (kind, aux, col, width):
      kind "loc"  - fully-live local 128-kv chunk; aux = first block of pair
      kind "pfx0" - leading chunk (lo, lo+1), live q-prefix [0, 64)
      kind "pfx1" - leading chunk (lo+2, lo+3), live q-prefix [0, 192)
      kind "stg"  - host-staged remote chunk (2 block slots); aux = stage idx
      kind "d1"   - diagonal half chunk (4t+2, 4t+3), q cols [128, 256)
    The per-core selection of remote blocks lives entirely in host data
    (KS/VS/RSTG/SMASK2), so the program is identical on all 8 cores.
    """
    slots = []
    col = 0

    def add(kind, aux, w):
        nonlocal col
        slots.append((kind, aux, col, w))
        col += w

    if t < 4:
        # full causal prefix: chunks (0,1) .. (4t, 4t+1) then half D1
        for i in range(2 * t + 1):
            add("loc", 2 * i, QTILE)
        add("d1", 4 * t + 2, 128)
    else:
        # all MASKED slots first (group 0, 896 cols: 64+192+256+256+128) so
        # their exp lands early and the DVE mask pass overlaps the interior
        # chunks' exp instead of sitting on the per-tile critical path
        lo = 4 * t - 16
        add("pfx0", lo, 64)
        add("pfx1", lo + 2, 192)
        add("stg", j * 4 + (t - 4), QTILE)
        add("loc", 4 * t, QTILE)          # D0 diagonal chunk
        add("d1", 4 * t + 2, 128)
        for i in range(6):
            add("loc", lo + 4 + 2 * i, QTILE)
    return slots


def _remote_class(core, j):
    """Blocks b with b % 8 == this value are remote-visible for head 4*core+j."""
    return (-(4 * core + j + 1)) % VSTRIDE


def _stage_blocks(core, j, t):
    """Remote blocks host-staged for (core, head-slot j, tile t>=4): all
    vertical-stride-selected blocks at or below the local window's leading
    staircase (b < 4t-12), at most 2."""
    r = _remote_class(core, j)
    return [b for b in range(4 * t - 12) if b % VSTRIDE == r]


_CACHE = {}


def _build_nc():
    import concourse.bacc as bacc
    import concourse.tile as tile
    from concourse import mybir

    dt = mybir.dt
    nc = bacc.Bacc(None)

    qT = nc.declare_dram_parameter("qT", [HPC * D, S], dt.bfloat16, isOutput=False)
    kT = nc.declare_dram_parameter("kT", [D, S], dt.bfloat16, isOutput=False)
    v1 = nc.declare_dram_parameter("v1", [D, NCHUNK * 129], dt.bfloat16, isOutput=False)
    m1x = nc.declare_dram_parameter("m1x", [D, 384], dt.bfloat16, isOutput=False)
    smask2 = nc.declare_dram_parameter("smask2", [D, HPC * 4 * 256], dt.bfloat16, isOutput=False)
    ks = nc.declare_dram_parameter("ks", [D, HPC * 4 * 128], dt.bfloat16, isOutput=False)
    vs = nc.declare_dram_parameter("vs", [D, HPC * 4 * 129], dt.bfloat16, isOutput=False)
    rstg = nc.declare_dram_parameter("rstg", [D, HPC * 4], dt.float32, isOutput=False)
    out = nc.declare_dram_parameter("out", [S, HPC * D], dt.float32, isOutput=True)

    EXP = mybir.ActivationFunctionType.Exp

    with tile.TileContext(nc) as tc:
        with (
            tc.tile_pool(name="consts", bufs=1) as consts,
            tc.tile_pool(name="ptp", bufs=3) as ptp,
            tc.tile_pool(name="ohp", bufs=2) as ohp,
            tc.tile_pool(name="lp", bufs=4) as lp,
            tc.tile_pool(name="stp", bufs=2, space="PSUM") as stp,
            tc.tile_pool(name="opp", bufs=2, space="PSUM") as opp,
        ):
            # warm the ACT exp table while input DMAs are in flight
            DUMI = consts.tile([128, 1], dt.float32, tag="dumi")
            DUMO = consts.tile([128, 1], dt.bfloat16, tag="dumo")
            nc.vector.memset(DUMI, 0.0)
            nc.scalar.activation(DUMO, DUMI, EXP, scale=1.0)



            # issue order matters: interleave small early pieces of KT/QT0/V1
            # so each successive tile's data lands just in time, instead of
            # serializing behind one big slow strided transfer
            KT = consts.tile([128, S], dt.bfloat16, tag="kt")
            QT = [consts.tile([128, S], dt.bfloat16, name=f"qt{h}", tag=f"qt{h}") for h in range(HPC)]
            V1 = consts.tile([128, NCHUNK * 129], dt.bfloat16, tag="v1")
            M1X = consts.tile([128, 384], dt.bfloat16, tag="m1x")
            nc.sync.dma_start(out=KT[:, 0:256], in_=kT[:, 0:256])
            nc.sync.dma_start(out=QT[0][:, 0:256], in_=qT[0:128, 0:256])
            nc.sync.dma_start(out=M1X, in_=m1x[:, :])
            nc.sync.dma_start(out=KT[:, 256:512], in_=kT[:, 256:512])
            nc.sync.dma_start(out=QT[0][:, 256:512], in_=qT[0:128, 256:512])
            nc.sync.dma_start(out=V1[:, 0:2 * 129], in_=v1[:, 0:2 * 129])
            nc.sync.dma_start(out=KT[:, 512:1024], in_=kT[:, 512:1024])
            nc.sync.dma_start(out=QT[0][:, 512:1024], in_=qT[0:128, 512:1024])
            nc.sync.dma_start(out=V1[:, 2 * 129:], in_=v1[:, 2 * 129:])
            nc.sync.dma_start(out=KT[:, 1024:S], in_=kT[:, 1024:S])
            nc.sync.dma_start(out=QT[0][:, 1024:S], in_=qT[0:128, 1024:S])
            RSTG = consts.tile([128, HPC * 4], dt.float32, tag="rstg")
            nc.sync.dma_start(out=RSTG, in_=rstg[:, :])
            KS = consts.tile([128, HPC * 4 * 128], dt.bfloat16, tag="ks")
            nc.sync.dma_start(out=KS, in_=ks[:, :])
            VS = consts.tile([128, HPC * 4 * 129], dt.bfloat16, tag="vs")
            nc.sync.dma_start(out=VS, in_=vs[:, :])
            SMASK2 = consts.tile([128, HPC * 4 * 256], dt.bfloat16, tag="smask2")
            nc.sync.dma_start(out=SMASK2[:, 0:1024], in_=smask2[:, 0:1024])
            for h in range(1, HPC):
                nc.sync.dma_start(out=QT[h], in_=qT[h * 128:(h + 1) * 128, :])
            nc.sync.dma_start(out=SMASK2[:, 1024:], in_=smask2[:, 1024:])

            def plan_scores(h, t, OH):
                """Build the slot plan + per-group score-matmul closures for
                tile (h, t). Returns (ctxt, group_emitters, mask_emitter).

                Slots pack greedily into 3-bank PSUM group tiles, one exp
                instruction per group. Leading staircase chunks are computed
                at their live causal prefix widths (64/192); remote blocks
                come from the host-staged KS/VS chunk."""
                slots = _tile_plan(h, t)
                qs = QT[h][:, t * QTILE:(t + 1) * QTILE]

                PT = ptp.tile([128, 10 * QTILE + 128 + 3 * QTILE], dt.bfloat16, tag="pt")

                # greedy-pack slots into PSUM group tiles of GROUP*QTILE f32.
                # For t>=4, force a boundary after the 5 masked slots (896
                # cols) so group 0 = masked slots, group 1 = interior chunks
                # (also keeps every 256-col matmul output bank-aligned).
                budget = GROUP * QTILE
                groups = []
                g_start = 0
                while g_start < len(slots):
                    g_end = g_start
                    used = 0
                    while g_end < len(slots) and used + slots[g_end][3] <= budget:
                        used += slots[g_end][3]
                        g_end += 1
                        if t >= 4 and g_end == 5:
                            break
                    groups.append((g_start, g_end, used))
                    g_start = g_end
                mask_gi = 0 if t >= 4 else len(groups) - 1

                def emit_group(gi):
                    g_start, g_end, used = groups[gi]
                    ST = stp.tile([128, budget], dt.float32, tag="st")
                    base = slots[g_start][2]
                    # wide streams first so each matmul's LDWEIGHTS hides
                    # under the previous matmul's rhs stream
                    ordered = sorted(slots[g_start:g_end], key=lambda sl: -sl[3])
                    for kind, aux, scol, w in ordered:
                        pcol = scol - base
                        if kind == "stg":
                            lhsT = KS[:, aux * 128: aux * 128 + 128]
                            rr = qs
                        else:
                            lhsT = KT[:, aux * BLK: aux * BLK + 128]
                            if kind == "loc":
                                rr = qs
                            elif kind == "pfx0":
                                rr = qs[:, 0:64]
                            elif kind == "pfx1":
                                rr = qs[:, 0:192]
                            else:  # d1: second q half only
                                rr = qs[:, 128:QTILE]
                        nc.tensor.matmul(
                            ST[:, pcol:pcol + w], lhsT=lhsT, rhs=rr,
                            start=True, stop=True,
                        )
                    nc.scalar.activation(
                        PT[:, base: base + used], ST[:, :used], EXP, scale=SCALE,
                    )

                def emit_masks():
                    # combined causal mask over the adjacent D0+D1 diagonal
                    # chunks (M1X = [tri(256) | tri(128)], 384 wide)
                    d0 = next(c for kk, aa, c, _w in slots
                              if kk == "loc" and aa == 4 * t)
                    nc.vector.tensor_mul(PT[:, d0:d0 + 384], PT[:, d0:d0 + 384], M1X)

                    if t >= 4:
                        # staircase+staging-exclusion mask over the two
                        # leading prefix chunks (64+192 = 256 cols)
                        mcol = (h * 4 + (t - 4)) * 256
                        nc.vector.tensor_mul(PT[:, 0:256], PT[:, 0:256],
                                             SMASK2[:, mcol:mcol + 256])
                        # staging chunk: zero dead block slots
                        kk, idx, scol, w = next(sl for sl in slots if sl[0] == "stg")
                        nc.vector.tensor_scalar_mul(
                            PT[:, scol:scol + w], PT[:, scol:scol + w],
                            RSTG[:, idx:idx + 1])

                ctxt = (h, t, slots, PT, OH)
                emitters = [lambda gi=gi: emit_group(gi) for gi in range(len(groups))]
                return ctxt, emitters, emit_masks, mask_gi

            def pv_half(ctxt, OP, s):
                _h, _t, slots, PT, _OH = ctxt
                # full-partition mms first so start=True covers all 128 rows;
                # the partial prefix mms (64 live q rows) accumulate after.
                fulls, partials = [], []
                for kind, aux, scol, w in slots:
                    if kind == "loc":
                        fulls.append((PT[:, scol + s * 128: scol + s * 128 + 128], 128,
                                      V1[:, (aux // 2) * 129: (aux // 2) * 129 + 129]))
                    elif kind == "stg":
                        fulls.append((PT[:, scol + s * 128: scol + s * 128 + 128], 128,
                                      VS[:, aux * 129: aux * 129 + 129]))
                    elif kind == "d1":
                        if s == 1:
                            fulls.append((PT[:, scol: scol + 128], 128,
                                          V1[:, (aux // 2) * 129: (aux // 2) * 129 + 129]))
                    elif kind == "pfx0":
                        if s == 0:
                            partials.append((PT[:, scol: scol + 64], 64,
                                             V1[:, (aux // 2) * 129: (aux // 2) * 129 + 129]))
                    elif kind == "pfx1":
                        v = V1[:, (aux // 2) * 129: (aux // 2) * 129 + 129]
                        if s == 0:
                            fulls.append((PT[:, scol: scol + 128], 128, v))
                        else:
                            partials.append((PT[:, scol + 128: scol + 192], 64, v))
                n = len(fulls) + len(partials)
                for mm, (lhsT, qw, rhs) in enumerate(fulls + partials):
                    o_ap = OP[:, s * 129:(s + 1) * 129] if qw == 128 \
                        else OP[0:qw, s * 129:(s + 1) * 129]
                    nc.tensor.matmul(o_ap, lhsT=lhsT, rhs=rhs,
                                     start=(mm == 0), stop=(mm == n - 1))

            def pv_norm(ctxt, OP):
                h, t, slots, PT, OH = ctxt
                # normalize: O / L (L = ones-column at col 128 of each half)
                LI = lp.tile([128, 2], dt.float32, tag="li")
                l_ap = OP[:, :].rearrange("p (s x) -> p s x", s=2)[:, :, 128]
                nc.vector.reciprocal(LI, l_ap)
                for s in range(2):
                    nc.vector.tensor_scalar_mul(
                        OH[:, 2 * t + s, :],
                        OP[:, s * 129: s * 129 + 128],
                        LI[:, s:s + 1],
                    )
                if h < HPC - 1:
                    store = {3: (0, 8), 5: (8, 12), 7: (12, 16)}.get(t)
                else:
                    # descending head: store fine-grained so the final DMA
                    # after the last tile is only 2 chunks
                    store = {4: (8, 16), 3: (6, 8), 2: (4, 6),
                             1: (2, 4), 0: (0, 2)}.get(t)
                if store is not None:
                    c0, c1 = store
                    nc.sync.dma_start(out=out_r[:, c0:c1, h * 128:(h + 1) * 128],
                                      in_=OH[:, c0:c1, :])

            out_r = out.rearrange("(c p) m -> p c m", p=128)  # [128, 16, 512]

            # Software pipeline, interleaved at group granularity: the PV
            # matmul halves of tile t-1 are emitted BETWEEN the score-group
            # matmuls of tile t, so the tensor engine always has work while
            # exp of the previous group runs, and exp is always one group
            # behind the matmuls that feed it.
            prev = None          # (ctxt, emit_masks) of previous tile
            for h in range(HPC):
                OH = ohp.tile([128, NCHUNK, 128], dt.float32, tag="oh")
                # last head runs t descending so the pipeline drains on the
                # smallest tile instead of the largest
                order = range(NT) if h < HPC - 1 else range(NT - 1, -1, -1)
                for t in order:
                    ctxt, gparts, masks, mask_gi = plan_scores(h, t, OH)
                    if prev is not None:
                        pctxt, pmasks = prev
                        OP = opp.tile([128, 2 * 129], dt.float32, tag="op")
                        pieces = [lambda: pv_half(pctxt, OP, 0),
                                  lambda: pv_half(pctxt, OP, 1),
                                  lambda: pv_norm(pctxt, OP)]
                    else:
                        pieces = []
                    # weave: g0 [masks] [pv0] g1 [pv1] ... leftovers
                    n = max(len(gparts), len(pieces))
                    for i in range(n):
                        if i < len(gparts):
                            gparts[i]()
                            if i == mask_gi:
                                masks()
                        if i < len(pieces):
                            pieces[i]()
                    prev = (ctxt, masks)
            pctxt, _ = prev
            OP = opp.tile([128, 2 * 129], dt.float32, tag="op")
            pv_half(pctxt, OP, 0)
            pv_half(pctxt, OP, 1)
            pv_norm(pctxt, OP)

    nc.finalize()
    return nc


def _host_inputs(query, key, value):
    """Build the 8 per-core input maps (host-side sharding + layout prep)."""
    q = np.asarray(query, dtype=np.float32)
    k = np.asarray(key, dtype=np.float32)
    v = np.asarray(value, dtype=np.float32)

    pp = np.arange(128)[:, None]
    qq = np.arange(QTILE)[None, :]
    m1x = np.concatenate([(qq >= pp).astype(np.float32),
                          (qq[:, :128] >= pp).astype(np.float32)],
                         axis=1).astype(BF16)              # [128, 384]

    in_maps = []
    for c in range(NCORES):
        qTc = np.ascontiguousarray(q[:, c * 512:(c + 1) * 512].T).astype(BF16)
        kTc = np.ascontiguousarray(k[:, c * D:(c + 1) * D].T).astype(BF16)
        vc = v[:, c * D:(c + 1) * D]                         # [2048, 128]
        vch = vc.reshape(NCHUNK, 128, D).transpose(1, 0, 2)  # [128, 16, 128]
        v1c = np.ones((128, NCHUNK, 129), dtype=np.float32)
        v1c[:, :, :128] = vch
        v1c = v1c.reshape(128, NCHUNK * 129).astype(BF16)

        # staging: per (head-slot j, tile t>=4) up to 2 vertical-stride
        # blocks packed into one 128-row chunk; dead slots zero + masked
        ksc = np.zeros((128, HPC, 4, 128), dtype=np.float32)
        vsc = np.ones((128, HPC, 4, 129), dtype=np.float32)
        vsc[:, :, :, :128] = 0.0
        rstgc = np.zeros((128, HPC * 4), dtype=np.float32)
        # leading staircase mask [128, HPC*4*256]: cols 0:64 = pfx0 chunk
        # (blocks lo, lo+1, q prefix 64), cols 64:256 = pfx1 (lo+2, lo+3,
        # prefix 192); remote-selected blocks excluded (staged instead)
        smask2c = np.zeros((128, HPC, 4, 256), dtype=np.float32)
        for j in range(HPC):
            r = _remote_class(c, j)
            for t in range(4, 8):
                lo = 4 * t - 16
                idx = j * 4 + (t - 4)
                for si, b in enumerate(_stage_blocks(c, j, t)):
                    ksc[:, j, t - 4, si * 64:(si + 1) * 64] = kTc[:, b * BLK:(b + 1) * BLK]
                    vsc[si * 64:(si + 1) * 64, j, t - 4, :128] = vch[
                        (b % 2) * 64:(b % 2) * 64 + 64, b // 2, :]
                    rstgc[si * 64:(si + 1) * 64, idx] = 1.0
                # pfx0: block lo (rows 0:64) never live locally; block lo+1
                # (rows 64:128) live for q < 64 unless staged
                if (lo + 1) % VSTRIDE != r:
                    smask2c[64:128, j, t - 4, 0:64] = 1.0
                # pfx1: block lo+2 rows live q < 128, lo+3 rows q < 192
                if (lo + 2) % VSTRIDE != r:
                    smask2c[0:64, j, t - 4, 64:192] = 1.0
                if (lo + 3) % VSTRIDE != r:
                    smask2c[64:128, j, t - 4, 64:256] = 1.0

        in_maps.append({
            "qT": qTc,
            "kT": kTc,
            "v1": v1c,
            "m1x": m1x,
            "smask2": smask2c.reshape(128, HPC * 4 * 256).astype(BF16),
            "ks": ksc.reshape(128, HPC * 4 * 128).astype(BF16),
            "vs": vsc.reshape(128, HPC * 4 * 129).astype(BF16),
            "rstg": rstgc,
        })
    return in_maps


def _get_nc():
    if "nc" not in _CACHE:
        _CACHE["nc"] = _build_nc()
    return _CACHE["nc"]


def kernel(query, key, value):
    from concourse.bass_utils import run_bass_kernel_spmd

    nc = _get_nc()
    in_maps = _host_inputs(query, key, value)
    res = run_bass_kernel_spmd(nc, in_maps, core_ids=list(range(NCORES)))
    outs = [res.results[c]["out"] for c in range(NCORES)]
    return np.concatenate(outs, axis=1).astype(np.float32)


if __name__ == "__main__":
    rng = np.random.default_rng(0)
    q = rng.standard_normal((S, NUM_HEADS * D), dtype=np.float32)
    k = rng.standard_normal((S, NUM_KV_HEADS * D), dtype=np.float32)
    v = rng.standard_normal((S, NUM_KV_HEADS * D), dtype=np.float32)
    o = kernel(query=q, key=k, value=v)
    print("kernel output", o.shape, o.dtype, np.abs(o).max())



# revision 28
# speedup vs baseline: 1.0400x; 1.0400x over previous
"""Block-sparse flash attention (local + vertical-stride pattern) on 8 TRN2
NeuronCores.

Sharding: tensor-parallel over heads. Core c gets q-heads [4c, 4c+4) and
kv-head c (the GQA group maps exactly: q-head h uses kv-head h//4). No
collectives needed; outputs are concatenated along the feature dim on host.

Per-core kernel (all shapes static, fully unrolled):
  - q is processed in tiles of 256 rows (4 sparse blocks of 64).
  - Scores are computed transposed, S^T[kv, q], per 128-wide kv chunk:
      matmul(out=S^T chunk, lhsT=K^T[:, kv_chunk], rhs=Q^T[:, q_tile])
    so softmax never needs a transpose of P for the PV matmul.
  - The window's leading staircase chunks are computed only at their live
    causal-prefix widths (64/192 q cols); remote (vertical-stride) blocks
    are packed by the HOST into per-(head,tile) staging chunks (KS/VS), so
    the SPMD program is core-independent and no wasted remote-parity
    columns are computed or exp'd.
  - exp (with the 1/sqrt(d) scale folded in) runs on the scalar engine
    straight out of PSUM into SBUF bf16, one instruction per PSUM group
    tile (6 kv chunks) so score matmuls of the next tile can reuse PSUM
    while exp of the previous group is still running.
  - Masking is multiplicative on P^T after exp: one combined causal mask
    for the adjacent D0+D1 diagonal chunks, a per-(head,tile) data mask
    for the staircase prefix chunks, and a 0/1 per-partition selector for
    dead staging slots.
  - PV accumulates O[q, d] with lhsT=P^T chunk and rhs=[V | 1]; the ones
    column makes the softmax denominator fall out of the same matmuls.
  - Emission is software-pipelined at group granularity: the PV matmul
    halves of tile t-1 are woven between the score-group matmuls of tile
    t, and within a group wide streams go first so every LDWEIGHTS hides
    under the previous matmul's rhs stream.
"""

import numpy as np
import ml_dtypes

BF16 = ml_dtypes.bfloat16

# Problem constants (hardcoded; see module docstring).
S = 2048
NUM_HEADS = 32
NUM_KV_HEADS = 8
D = 128
BLK = 64
LOCAL_BLOCKS = 16
VSTRIDE = 8
SCALE = 0.08838834764831845
NCORES = 8
HPC = NUM_HEADS // NCORES          # heads per core = 4
QTILE = 256                        # q rows per tile (4 sparse blocks)
NT = S // QTILE                    # 8 tiles
NCHUNK = S // 128                  # 16 kv chunks of 128
GROUP = 6                          # kv chunks per PSUM group tile (3 banks)


def _tile_plan(j, t):
    """Static slot plan for head-slot j (0..3), q-tile t. Core-independent.

    Returns a list of slots (kind, aux, col, width):
      kind "loc"  - fully-live local 128-kv chunk; aux = first block of pair
      kind "pfx0" - leading chunk (lo, lo+1), live q-prefix [0, 64)
      kind "pfx1" - leading chunk (lo+2, lo+3), live q-prefix [0, 192)
      kind "stg"  - host-staged remote chunk (2 block slots); aux = stage idx
      kind "d1"   - diagonal half chunk (4t+2, 4t+3), q cols [128, 256)
    The per-core selection of remote blocks lives entirely in host data
    (KS/VS/RSTG/SMASK2), so the program is identical on all 8 cores.
    """
    slots = []
    col = 0

    def add(kind, aux, w):
        nonlocal col
        slots.append((kind, aux, col, w))
        col += w

    if t < 4:
        # full causal prefix: chunks (0,1) .. (4t, 4t+1) then half D1
        for i in range(2 * t + 1):
            add("loc", 2 * i, QTILE)
        add("d1", 4 * t + 2, 128)
    else:
        lo = 4 * t - 16
        add("pfx0", lo, 64)
        add("pfx1", lo + 2, 192)
        for i in range(6):
            add("loc", lo + 4 + 2 * i, QTILE)
        add("stg", j * 4 + (t - 4), QTILE)
        add("loc", 4 * t, QTILE)          # D0 diagonal chunk
        add("d1", 4 * t + 2, 128)
    return slots


def _remote_class(core, j):
    """Blocks b with b % 8 == this value are remote-visible for head 4*core+j."""
    return (-(4 * core + j + 1)) % VSTRIDE


def _stage_blocks(core, j, t):
    """Remote blocks host-staged for (core, head-slot j, tile t>=4): all
    vertical-stride-selected blocks at or below the local window's leading
    staircase (b < 4t-12), at most 2."""
    r = _remote_class(core, j)
    return [b for b in range(4 * t - 12) if b % VSTRIDE == r]


_CACHE = {}


def _build_nc():
    import concourse.bacc as bacc
    import concourse.tile as tile
    from concourse import mybir

    dt = mybir.dt
    nc = bacc.Bacc(None)

    qT = nc.declare_dram_parameter("qT", [HPC * D, S], dt.bfloat16, isOutput=False)
    kT = nc.declare_dram_parameter("kT", [D, S], dt.bfloat16, isOutput=False)
    v1 = nc.declare_dram_parameter("v1", [D, NCHUNK * 129], dt.bfloat16, isOutput=False)
    m1x = nc.declare_dram_parameter("m1x", [D, 384], dt.bfloat16, isOutput=False)
    smask2 = nc.declare_dram_parameter("smask2", [D, HPC * 4 * 256], dt.bfloat16, isOutput=False)
    ks = nc.declare_dram_parameter("ks", [D, HPC * 4 * 128], dt.bfloat16, isOutput=False)
    vs = nc.declare_dram_parameter("vs", [D, HPC * 4 * 129], dt.bfloat16, isOutput=False)
    rstg = nc.declare_dram_parameter("rstg", [D, HPC * 4], dt.float32, isOutput=False)
    out = nc.declare_dram_parameter("out", [S, HPC * D], dt.float32, isOutput=True)

    EXP = mybir.ActivationFunctionType.Exp

    with tile.TileContext(nc) as tc:
        with (
            tc.tile_pool(name="consts", bufs=1) as consts,
            tc.tile_pool(name="ptp", bufs=3) as ptp,
            tc.tile_pool(name="ohp", bufs=2) as ohp,
            tc.tile_pool(name="lp", bufs=4) as lp,
            tc.tile_pool(name="stp", bufs=2, space="PSUM") as stp,
            tc.tile_pool(name="opp", bufs=2, space="PSUM") as opp,
        ):
            # warm the ACT exp table while input DMAs are in flight
            DUMI = consts.tile([128, 1], dt.float32, tag="dumi")
            DUMO = consts.tile([128, 1], dt.bfloat16, tag="dumo")
            nc.vector.memset(DUMI, 0.0)
            nc.scalar.activation(DUMO, DUMI, EXP, scale=1.0)



            # issue order matters: interleave small early pieces of KT/QT0/V1
            # so each successive tile's data lands just in time, instead of
            # serializing behind one big slow strided transfer
            KT = consts.tile([128, S], dt.bfloat16, tag="kt")
            QT = [consts.tile([128, S], dt.bfloat16, name=f"qt{h}", tag=f"qt{h}") for h in range(HPC)]
            V1 = consts.tile([128, NCHUNK * 129], dt.bfloat16, tag="v1")
            M1X = consts.tile([128, 384], dt.bfloat16, tag="m1x")
            nc.sync.dma_start(out=KT[:, 0:256], in_=kT[:, 0:256])
            nc.sync.dma_start(out=QT[0][:, 0:256], in_=qT[0:128, 0:256])
            nc.sync.dma_start(out=M1X, in_=m1x[:, :])
            nc.sync.dma_start(out=KT[:, 256:512], in_=kT[:, 256:512])
            nc.sync.dma_start(out=QT[0][:, 256:512], in_=qT[0:128, 256:512])
            nc.sync.dma_start(out=V1[:, 0:2 * 129], in_=v1[:, 0:2 * 129])
            nc.sync.dma_start(out=KT[:, 512:1024], in_=kT[:, 512:1024])
            nc.sync.dma_start(out=QT[0][:, 512:1024], in_=qT[0:128, 512:1024])
            nc.sync.dma_start(out=V1[:, 2 * 129:], in_=v1[:, 2 * 129:])
            nc.sync.dma_start(out=KT[:, 1024:S], in_=kT[:, 1024:S])
            nc.sync.dma_start(out=QT[0][:, 1024:S], in_=qT[0:128, 1024:S])
            RSTG = consts.tile([128, HPC * 4], dt.float32, tag="rstg")
            nc.sync.dma_start(out=RSTG, in_=rstg[:, :])
            KS = consts.tile([128, HPC * 4 * 128], dt.bfloat16, tag="ks")
            nc.sync.dma_start(out=KS, in_=ks[:, :])
            VS = consts.tile([128, HPC * 4 * 129], dt.bfloat16, tag="vs")
            nc.sync.dma_start(out=VS, in_=vs[:, :])
            SMASK2 = consts.tile([128, HPC * 4 * 256], dt.bfloat16, tag="smask2")
            nc.sync.dma_start(out=SMASK2[:, 0:1024], in_=smask2[:, 0:1024])
            for h in range(1, HPC):
                nc.sync.dma_start(out=QT[h], in_=qT[h * 128:(h + 1) * 128, :])
            nc.sync.dma_start(out=SMASK2[:, 1024:], in_=smask2[:, 1024:])

            def plan_scores(h, t, OH):
                """Build the slot plan + per-group score-matmul closures for
                tile (h, t). Returns (ctxt, group_emitters, mask_emitter).

                Slots pack greedily into 3-bank PSUM group tiles, one exp
                instruction per group. Leading staircase chunks are computed
                at their live causal prefix widths (64/192); remote blocks
                come from the host-staged KS/VS chunk."""
                slots = _tile_plan(h, t)
                qs = QT[h][:, t * QTILE:(t + 1) * QTILE]

                PT = ptp.tile([128, 10 * QTILE + 128 + 3 * QTILE], dt.bfloat16, tag="pt")

                # greedy-pack slots into PSUM group tiles of GROUP*QTILE f32
                budget = GROUP * QTILE
                groups = []
                g_start = 0
                while g_start < len(slots):
                    g_end = g_start
                    used = 0
                    while g_end < len(slots) and used + slots[g_end][3] <= budget:
                        used += slots[g_end][3]
                        g_end += 1
                    groups.append((g_start, g_end, used))
                    g_start = g_end
                mask_gi = len(groups) - 1

                def emit_group(gi):
                    g_start, g_end, used = groups[gi]
                    ST = stp.tile([128, budget], dt.float32, tag="st")
                    base = slots[g_start][2]
                    # wide streams first so each matmul's LDWEIGHTS hides
                    # under the previous matmul's rhs stream
                    ordered = sorted(slots[g_start:g_end], key=lambda sl: -sl[3])
                    for kind, aux, scol, w in ordered:
                        pcol = scol - base
                        if kind == "stg":
                            lhsT = KS[:, aux * 128: aux * 128 + 128]
                            rr = qs
                        else:
                            lhsT = KT[:, aux * BLK: aux * BLK + 128]
                            if kind == "loc":
                                rr = qs
                            elif kind == "pfx0":
                                rr = qs[:, 0:64]
                            elif kind == "pfx1":
                                rr = qs[:, 0:192]
                            else:  # d1: second q half only
                                rr = qs[:, 128:QTILE]
                        nc.tensor.matmul(
                            ST[:, pcol:pcol + w], lhsT=lhsT, rhs=rr,
                            start=True, stop=True,
                        )
                    nc.scalar.activation(
                        PT[:, base: base + used], ST[:, :used], EXP, scale=SCALE,
                    )

                def emit_masks():
                    # combined causal mask over the adjacent D0+D1 diagonal
                    # chunks (M1X = [tri(256) | tri(128)], 384 wide)
                    d0 = next(c for kk, aa, c, _w in slots
                              if kk == "loc" and aa == 4 * t)
                    nc.vector.tensor_mul(PT[:, d0:d0 + 384], PT[:, d0:d0 + 384], M1X)

                    if t >= 4:
                        # staircase+staging-exclusion mask over the two
                        # leading prefix chunks (64+192 = 256 cols)
                        mcol = (h * 4 + (t - 4)) * 256
                        nc.vector.tensor_mul(PT[:, 0:256], PT[:, 0:256],
                                             SMASK2[:, mcol:mcol + 256])
                        # staging chunk: zero dead block slots
                        kk, idx, scol, w = next(sl for sl in slots if sl[0] == "stg")
                        nc.vector.tensor_scalar_mul(
                            PT[:, scol:scol + w], PT[:, scol:scol + w],
                            RSTG[:, idx:idx + 1])

                ctxt = (h, t, slots, PT, OH)
                emitters = [lambda gi=gi: emit_group(gi) for gi in range(len(groups))]
                return ctxt, emitters, emit_masks, mask_gi

            def pv_half(ctxt, OP, s):
                _h, _t, slots, PT, _OH = ctxt
                # full-partition mms first so start=True covers all 128 rows;
                # the partial prefix mms (64 live q rows) accumulate after.
                fulls, partials = [], []
                for kind, aux, scol, w in slots:
                    if kind == "loc":
                        fulls.append((PT[:, scol + s * 128: scol + s * 128 + 128], 128,
                                      V1[:, (aux // 2) * 129: (aux // 2) * 129 + 129]))
                    elif kind == "stg":
                        fulls.append((PT[:, scol + s * 128: scol + s * 128 + 128], 128,
                                      VS[:, aux * 129: aux * 129 + 129]))
                    elif kind == "d1":
                        if s == 1:
                            fulls.append((PT[:, scol: scol + 128], 128,
                                          V1[:, (aux // 2) * 129: (aux // 2) * 129 + 129]))
                    elif kind == "pfx0":
                        if s == 0:
                            partials.append((PT[:, scol: scol + 64], 64,
                                             V1[:, (aux // 2) * 129: (aux // 2) * 129 + 129]))
                    elif kind == "pfx1":
                        v = V1[:, (aux // 2) * 129: (aux // 2) * 129 + 129]
                        if s == 0:
                            fulls.append((PT[:, scol: scol + 128], 128, v))
                        else:
                            partials.append((PT[:, scol + 128: scol + 192], 64, v))
                n = len(fulls) + len(partials)
                for mm, (lhsT, qw, rhs) in enumerate(fulls + partials):
                    o_ap = OP[:, s * 129:(s + 1) * 129] if qw == 128 \
                        else OP[0:qw, s * 129:(s + 1) * 129]
                    nc.tensor.matmul(o_ap, lhsT=lhsT, rhs=rhs,
                                     start=(mm == 0), stop=(mm == n - 1))

            def pv_norm(ctxt, OP):
                h, t, slots, PT, OH = ctxt
                # normalize: O / L (L = ones-column at col 128 of each half)
                LI = lp.tile([128, 2], dt.float32, tag="li")
                l_ap = OP[:, :].rearrange("p (s x) -> p s x", s=2)[:, :, 128]
                nc.vector.reciprocal(LI, l_ap)
                for s in range(2):
                    nc.vector.tensor_scalar_mul(
                        OH[:, 2 * t + s, :],
                        OP[:, s * 129: s * 129 + 128],
                        LI[:, s:s + 1],
                    )
                if h < HPC - 1:
                    store = {3: (0, 8), 5: (8, 12), 7: (12, 16)}.get(t)
                else:
                    # descending head: store fine-grained so the final DMA
                    # after the last tile is only 2 chunks
                    store = {4: (8, 16), 3: (6, 8), 2: (4, 6),
                             1: (2, 4), 0: (0, 2)}.get(t)
                if store is not None:
                    c0, c1 = store
                    nc.sync.dma_start(out=out_r[:, c0:c1, h * 128:(h + 1) * 128],
                                      in_=OH[:, c0:c1, :])

            out_r = out.rearrange("(c p) m -> p c m", p=128)  # [128, 16, 512]

            # Software pipeline, interleaved at group granularity: the PV
            # matmul halves of tile t-1 are emitted BETWEEN the score-group
            # matmuls of tile t, so the tensor engine always has work while
            # exp of the previous group runs, and exp is always one group
            # behind the matmuls that feed it.
            prev = None          # (ctxt, emit_masks) of previous tile
            for h in range(HPC):
                OH = ohp.tile([128, NCHUNK, 128], dt.float32, tag="oh")
                # last head runs t descending so the pipeline drains on the
                # smallest tile instead of the largest
                order = range(NT) if h < HPC - 1 else range(NT - 1, -1, -1)
                for t in order:
                    ctxt, gparts, masks, mask_gi = plan_scores(h, t, OH)
                    if prev is not None:
                        pctxt, pmasks = prev
                        OP = opp.tile([128, 2 * 129], dt.float32, tag="op")
                        pieces = [lambda: pv_half(pctxt, OP, 0),
                                  lambda: pv_half(pctxt, OP, 1),
                                  lambda: pv_norm(pctxt, OP)]
                    else:
                        pieces = []
                    # weave: g0 [masks] [pv0] g1 [pv1] ... leftovers
                    n = max(len(gparts), len(pieces))
                    for i in range(n):
                        if i < len(gparts):
                            gparts[i]()
                            if i == mask_gi:
                                masks()
                        if i < len(pieces):
                            pieces[i]()
                    prev = (ctxt, masks)
            pctxt, _ = prev
            OP = opp.tile([128, 2 * 129], dt.float32, tag="op")
            pv_half(pctxt, OP, 0)
            pv_half(pctxt, OP, 1)
            pv_norm(pctxt, OP)

    nc.finalize()
    return nc


def _host_inputs(query, key, value):
    """Build the 8 per-core input maps (host-side sharding + layout prep)."""
    q = np.asarray(query, dtype=np.float32)
    k = np.asarray(key, dtype=np.float32)
    v = np.asarray(value, dtype=np.float32)

    pp = np.arange(128)[:, None]
    qq = np.arange(QTILE)[None, :]
    m1x = np.concatenate([(qq >= pp).astype(np.float32),
                          (qq[:, :128] >= pp).astype(np.float32)],
                         axis=1).astype(BF16)              # [128, 384]

    in_maps = []
    for c in range(NCORES):
        qTc = np.ascontiguousarray(q[:, c * 512:(c + 1) * 512].T).astype(BF16)
        kTc = np.ascontiguousarray(k[:, c * D:(c + 1) * D].T).astype(BF16)
        vc = v[:, c * D:(c + 1) * D]                         # [2048, 128]
        vch = vc.reshape(NCHUNK, 128, D).transpose(1, 0, 2)  # [128, 16, 128]
        v1c = np.ones((128, NCHUNK, 129), dtype=np.float32)
        v1c[:, :, :128] = vch
        v1c = v1c.reshape(128, NCHUNK * 129).astype(BF16)

        # staging: per (head-slot j, tile t>=4) up to 2 vertical-stride
        # blocks packed into one 128-row chunk; dead slots zero + masked
        ksc = np.zeros((128, HPC, 4, 128), dtype=np.float32)
        vsc = np.ones((128, HPC, 4, 129), dtype=np.float32)
        vsc[:, :, :, :128] = 0.0
        rstgc = np.zeros((128, HPC * 4), dtype=np.float32)
        # leading staircase mask [128, HPC*4*256]: cols 0:64 = pfx0 chunk
        # (blocks lo, lo+1, q prefix 64), cols 64:256 = pfx1 (lo+2, lo+3,
        # prefix 192); remote-selected blocks excluded (staged instead)
        smask2c = np.zeros((128, HPC, 4, 256), dtype=np.float32)
        for j in range(HPC):
            r = _remote_class(c, j)
            for t in range(4, 8):
                lo = 4 * t - 16
                idx = j * 4 + (t - 4)
                for si, b in enumerate(_stage_blocks(c, j, t)):
                    ksc[:, j, t - 4, si * 64:(si + 1) * 64] = kTc[:, b * BLK:(b + 1) * BLK]
                    vsc[si * 64:(si + 1) * 64, j, t - 4, :128] = vch[
                        (b % 2) * 64:(b % 2) * 64 + 64, b // 2, :]
                    rstgc[si * 64:(si + 1) * 64, idx] = 1.0
                # pfx0: block lo (rows 0:64) never live locally; block lo+1
                # (rows 64:128) live for q < 64 unless staged
                if (lo + 1) % VSTRIDE != r:
                    smask2c[64:128, j, t - 4, 0:64] = 1.0
                # pfx1: block lo+2 rows live q < 128, lo+3 rows q < 192
                if (lo + 2) % VSTRIDE != r:
                    smask2c[0:64, j, t - 4, 64:192] = 1.0
                if (lo + 3) % VSTRIDE != r:
                    smask2c[64:128, j, t - 4, 64:256] = 1.0

        in_maps.append({
            "qT": qTc,
            "kT": kTc,
            "v1": v1c,
            "m1x": m1x,
            "smask2": smask2c.reshape(128, HPC * 4 * 256).astype(BF16),
            "ks": ksc.reshape(128, HPC * 4 * 128).astype(BF16),
            "vs": vsc.reshape(128, HPC * 4 * 129).astype(BF16),
            "rstg": rstgc,
        })
    return in_maps


def _get_nc():
    if "nc" not in _CACHE:
        _CACHE["nc"] = _build_nc()
    return _CACHE["nc"]


def kernel(query, key, value):
    from concourse.bass_utils import run_bass_kernel_spmd

    nc = _get_nc()
    in_maps = _host_inputs(query, key, value)
    res = run_bass_kernel_spmd(nc, in_maps, core_ids=list(range(NCORES)))
    outs = [res.results[c]["out"] for c in range(NCORES)]
    return np.concatenate(outs, axis=1).astype(np.float32)


if __name__ == "__main__":
    rng = np.random.default_rng(0)
    q = rng.standard_normal((S, NUM_HEADS * D), dtype=np.float32)
    k = rng.standard_normal((S, NUM_KV_HEADS * D), dtype=np.float32)
    v = rng.standard_normal((S, NUM_KV_HEADS * D), dtype=np.float32)
    o = kernel(query=q, key=k, value=v)
    print("kernel output", o.shape, o.dtype, np.abs(o).max())

